# revision 9
# baseline (speedup 1.0000x reference)
"""Trainium2 Bass kernel for nn_DisOrFuncf_34067680591904.

Mathematical note: the reference's output *value* is exactly
fout = sigmoid(MLP(x[:, 0, :])) — the inner/GOGradX machinery only
shapes gradients.  The MLP is 784 -> 512 -> 256 -> 1 with leaky-relu
(0.2) and sigmoid.  Eval path (is_train_g == 0) applies the same MLP to
every (batch, level) row.

Strategy: data parallel — 32 rows/core (train) or 128 (eval); weights
replicated, quantized to fp8-e4m3 (measured end-to-end max rel err
4.5e-3 vs the fp32 reference, ~4x inside the 2e-2 gate).  Scales keep
fp8 values in the normal range: W1*8, W2*4, W3*8; leaky-relu is
positively homogeneous so scales fold into the final sigmoid's `scale`.

Transposed dataflow (d1/d2 kept as [feature, batch]):
  L1  ps1_j[128,R] += w1(j,c).T @ xt_c   j=h1-chunk(4), c=k-chunk(6)
      + bf16 tail chunk (features 768:784 + b1 ones-row)
  lrelu1 on DVE (scalar_mul + max), cast d1t to fp8
  L2  ps2_{j2}[128,R] += w2(j,j2).T @ d1t_j       (fp8)
  lrelu2 + b2 bias on DVE: t=0.2*(ps2+b2); d2=max(ps2+b2, t) -> bf16
  L3  ps3[1,R] += w3_col.T @ d2t_slice (bf16; single-partition output
      row keeps the final 128B store on one DMA engine)
  sigmoid on ACT with scale=1/256, bias=b3

DMA layout (perfetto-driven): the two HWDGE queues stream ~90-200 GB/s
each, so the bulk fp8 bytes are split into one large transfer per
queue; the tiny bf16 tail/const tiles ride the SWDGE (gpsimd) queue
and land early so no accumulation group waits on them.
  sync  : fa = [xt | w1_j0 | w1_j1 | w2_a]     (~287 KB)
  scalar: fb = [w1_j2 | w1_j3 | w2_b]          (~262 KB)
  gpsimd: tl = [w1t | xtt] (18.5 KB), cst (1.3 KB)
"""

import os as _os

import numpy as np
import ml_dtypes

N_CORES = 8
BATCH, NC_LVL, D_IN, D_H1, D_H2 = 256, 4, 784, 512, 256
N_WARM = int(_os.environ.get("KERNEL_N_WARM", "4"))

_compiled = {}  # rows_per_core -> nc


def _build_nc(R: int):
    import concourse.bacc as bacc
    import concourse.tile as tile
    from concourse import mybir

    f32 = mybir.dt.float32
    bf16 = mybir.dt.bfloat16
    fp8 = mybir.dt.float8e4
    nc = bacc.Bacc("TRN2", target_bir_lowering=False, debug=False,
                   num_devices=N_CORES, enable_partition_id=False)

    # Three-stage bulk ladder, all on the sync HWDGE queue (the SDMA
    # engines drain one queue FIFO, so ordering beats queue-splitting):
    #   fa1: [xt | w1_j0 | tl bytes | cst | cstf | b2r bytes]
    #   fa2: [w1_j1 | w1_j2]     fa3: [w1_j3 | w2]
    T0 = 6 * R + 768                     # tl: [17, 512+R] bf16 as bytes
    C0 = T0 + 2 * (512 + R)              # cst: [128, 2] bf16 as bytes
    F0 = C0 + 4                          # cstf: [128, 3] f32 as bytes
    B0 = F0 + 12                         # b2r: [1, 256] bf16 as bytes
    FA = B0 + 512
    assert T0 % 4 == 0 and C0 % 4 == 0 and F0 % 4 == 0 and B0 % 4 == 0
    fa_d = nc.dram_tensor("fa", [128, FA], fp8, kind="ExternalInput")
    fb_d = nc.dram_tensor("fb", [128, 1536], fp8, kind="ExternalInput")
    fc_d = nc.dram_tensor("fc", [128, 1792], fp8, kind="ExternalInput")
    out_d = nc.dram_tensor("out", [1, R], f32, kind="ExternalOutput")

    with tile.TileContext(nc) as tc:
        with (
            tc.tile_pool(name="const", bufs=1) as cpool,
            tc.tile_pool(name="work", bufs=2) as wpool,
            tc.tile_pool(name="psum", bufs=1, space="PSUM") as ppool,
        ):
            # ---- PE warm-up: keep the HAM clock gate open while DMAs
            # stream (bf16 dummy matmuls on a memset tile).
            if N_WARM:
                wa = cpool.tile([128, 128], bf16, tag="warm_a")
                nc.vector.memset(wa[:], 0.0)
                psw = ppool.tile([128, 128], f32, tag="psw")
                for i in range(N_WARM):
                    nc.tensor.matmul(psw[:], wa[:], wa[:],
                                     start=(i == 0), stop=(i == N_WARM - 1))
                wsb = cpool.tile([1, 1], f32, tag="wsb")
                nc.vector.tensor_copy(wsb[:], psw[0:1, 0:1])

            # ---- DMAs ----
            fa = cpool.tile([128, FA], fp8, tag="fa")
            nc.sync.dma_start(out=fa[:], in_=fa_d[:])
            fb = cpool.tile([128, 1536], fp8, tag="fb")
            nc.sync.dma_start(out=fb[:], in_=fb_d[:])
            fc = cpool.tile([128, 1792], fp8, tag="fc")
            nc.sync.dma_start(out=fc[:], in_=fc_d[:])

            xt = fa[:, 0:6 * R]
            w1 = [fa[:, 6 * R:6 * R + 768], fb[:, 0:768],
                  fb[:, 768:1536], fc[:, 0:768]]
            w2s = fc[:, 768:1792]
            tlball = fa[0:17, T0:T0 + 2 * (512 + R)].bitcast(bf16)
            w1t = tlball[:, 0:512]
            xtt = tlball[:, 512:512 + R]
            cst = fa[:, C0:C0 + 4].bitcast(bf16)
            cstf = fa[:, F0:F0 + 12].bitcast(f32)
            b2r = fa[0:1, B0:B0 + 512].bitcast(bf16)
            ones1 = cpool.tile([1, R], bf16, tag="ones1")
            nc.vector.memset(ones1[:], 1.0)

            def w2(j, j2):
                return w2s[:, 256 * j + 128 * j2:256 * j + 128 * j2 + 128]

            # ---- PSUM tiles ----
            ps1 = [ppool.tile([128, R], f32, tag=f"ps1_{j}", name=f"ps1_{j}")
                   for j in range(4)]
            ps2 = [ppool.tile([128, R], f32, tag=f"ps2_{j2}", name=f"ps2_{j2}")
                   for j2 in range(2)]
            ps3 = ppool.tile([1, R], f32, tag="ps3")

            d1t = [None] * 4

            def l1_chunk(j):
                for c in range(6):
                    nc.tensor.matmul(ps1[j][:],
                                     w1[j][:, 128 * c:128 * c + 128],
                                     xt[:, R * c:R * c + R],
                                     start=(c == 0), stop=False)
                nc.tensor.matmul(ps1[j][:],
                                 w1t[:, 128 * j:128 * j + 128],
                                 xtt[:, 0:R],
                                 start=False, stop=True)

            def lrelu1(j):
                # 0.2x on ACT (exact Copy datapath), max on DVE
                t = wpool.tile([128, R], f32, tag="t1")
                nc.scalar.activation(t[:], ps1[j][:],
                                     mybir.ActivationFunctionType.Copy,
                                     scale=0.2)
                d = cpool.tile([128, R], fp8, tag=f"d1t_{j}",
                               name=f"d1t_{j}")
                nc.vector.tensor_max(d[:], ps1[j][:], t[:])
                d1t[j] = d

            def l2_chunk(j):
                for j2 in range(2):
                    nc.tensor.matmul(ps2[j2][:], w2(j, j2), d1t[j][:],
                                     start=(j == 0), stop=False)
                if j == 3:
                    # b2 bias rides K=1 ones matmuls closing each group
                    for j2 in range(2):
                        nc.tensor.matmul(ps2[j2][:],
                                         b2r[0:1, 128 * j2:128 * j2 + 128],
                                         ones1[:], start=False, stop=True)

            # PE program order: interleave L1/L2 so the PE never waits
            # on the DVE lrelu of the chunk it just produced.
            l1_chunk(0)
            l1_chunk(1)
            lrelu1(0)
            lrelu1(1)
            l1_chunk(2)
            l2_chunk(0)
            lrelu1(2)
            l1_chunk(3)
            l2_chunk(1)
            lrelu1(3)
            l2_chunk(2)
            l2_chunk(3)

            # ---- L2 lrelu (bias already accumulated in PSUM) ----
            d2t = cpool.tile([128, 2 * R], bf16, tag="d2t")
            for j2 in range(2):
                t = wpool.tile([128, R], f32, tag="t2")
                nc.scalar.activation(t[:], ps2[j2][:],
                                     mybir.ActivationFunctionType.Copy,
                                     scale=0.2)
                nc.vector.tensor_max(d2t[:, R * j2:R * j2 + R],
                                     ps2[j2][:], t[:])

            # ---- L3: ps3[1,R] = sum_o w3[o] * d2t[o,b] ----
            nc.tensor.matmul(ps3[:], cst[:, 0:1], d2t[:, 0:R],
                             start=True, stop=False)
            nc.tensor.matmul(ps3[:], cst[:, 1:2], d2t[:, R:2 * R],
                             start=False, stop=True)

            # ---- sigmoid((1/256) * ps3 + b3) on ACT ----
            ob = cpool.tile([1, R], f32, tag="ob")
            nc.scalar.activation(ob[:], ps3[:],
                                 mybir.ActivationFunctionType.Sigmoid,
                                 bias=cstf[0:1, 0:1], scale=1.0 / 256.0)
            nc.scalar.dma_start(out=out_d[:], in_=ob[:])

    nc.compile()
    return nc


def _get_nc(R: int):
    if R not in _compiled:
        _compiled[R] = _build_nc(R)
    return _compiled[R]


def _pack_weights(W1, b1, W2, b2, W3, b3):
    f = np.float32
    bf = ml_dtypes.bfloat16
    e4 = ml_dtypes.float8_e4m3
    # w1_j[p, 128c + m] = 8*W1[128j + m, 128c + p]
    w1s = (8.0 * W1).astype(f)
    w1js = []
    for j in range(4):
        blk = w1s[128 * j:128 * j + 128, :768].reshape(128, 6, 128)
        w1js.append(np.ascontiguousarray(
            blk.transpose(2, 1, 0).reshape(128, 768)).astype(e4))
    # tl: [w1t | xtt]; w1t[p, 128j + m] = 8*W1[128j + m, 768 + p],
    # row 16 = 8*b1
    w1t = np.empty((17, 512), dtype=bf)
    w1t[:16] = w1s[:, 768:784].T
    w1t[16] = (8.0 * b1).astype(f)
    # w2 [p, 256j + 128j2 + m] = 4*W2[128j2 + m, 128j + p]
    w2s = (4.0 * W2).astype(f)
    w2p = np.empty((128, 1024), dtype=e4)
    for j in range(4):
        for j2 in range(2):
            w2p[:, 256 * j + 128 * j2:256 * j + 128 * j2 + 128] = \
                w2s[128 * j2:128 * j2 + 128, 128 * j:128 * j + 128].T
    fb = np.concatenate([w1js[1], w1js[2]], axis=1)
    fc = np.concatenate([w1js[3], w2p], axis=1)
    # cst cols: 0,1 = 8*w3 (bf16); cstf col 0 = b3 (f32); b2r = 32*b2 bf16
    cst = np.empty((128, 2), dtype=bf)
    cst[:, 0] = (8.0 * W3[0, :128]).astype(f)
    cst[:, 1] = (8.0 * W3[0, 128:]).astype(f)
    cstf = np.empty((128, 3), dtype=f)
    cstf[:, 0] = b3[0]
    cstf[:, 1] = 0.0
    cstf[:, 2] = 0.0
    b2r = (32.0 * b2).astype(bf).reshape(1, 256)
    return w1js, w1t, fb, fc, cst, cstf, b2r


def _pack_x(rows_c: np.ndarray, R: int, w1js, w1t, cst, cstf, b2r):
    # fa = [xt | w1_j0 | tl bytes | cst | cstf | b2r bytes]
    # xt[p, cR + b] = x[b, 128c + p] fp8
    # tl = [w1t | xtt]; xtt rows 0:16 = x[:, 768:784].T, row 16 = ones
    e4 = ml_dtypes.float8_e4m3
    bf = ml_dtypes.bfloat16
    T0 = 6 * R + 768
    C0 = T0 + 2 * (512 + R)
    F0 = C0 + 4
    B0 = F0 + 12
    FA = B0 + 512
    xt = np.ascontiguousarray(
        rows_c[:, :768].reshape(R, 6, 128).transpose(2, 1, 0)
        .reshape(128, 6 * R)).astype(e4)
    fa = np.zeros((128, FA), dtype=e4)
    fa[:, :6 * R] = xt
    fa[:, 6 * R:T0] = w1js[0]
    tl = np.empty((17, 512 + R), dtype=bf)
    tl[:, :512] = w1t
    tl[:16, 512:] = rows_c[:, 768:784].T
    tl[16, 512:] = 1.0
    u8 = fa.view(np.uint8)
    u8[0:17, T0:C0] = tl.view(np.uint8)
    u8[:, C0:F0] = cst.view(np.uint8)
    u8[:, F0:B0] = cstf.view(np.uint8)
    u8[0:1, B0:FA] = b2r.view(np.uint8)
    return fa


_trace_opts = None   # test harness hook: kwargs for run_bass_kernel_spmd
_last_results = None


def _run(rows: np.ndarray, R: int, weights) -> np.ndarray:
    global _last_results
    import time
    from concourse.bass_utils import run_bass_kernel_spmd

    nc = _get_nc(R)
    w1js, w1t, fb, fc, cst, cstf, b2r = weights
    in_maps = []
    for c in range(N_CORES):
        fa = _pack_x(rows[c * R:(c + 1) * R], R, w1js, w1t, cst, cstf, b2r)
        in_maps.append({"fa": fa, "fb": fb, "fc": fc})
    last_exc = None
    for attempt in range(4):
        try:
            res = run_bass_kernel_spmd(nc, in_maps, list(range(N_CORES)),
                                       **(_trace_opts or {}))
            break
        except Exception as e:  # transient device wedge: wait and retry
            last_exc = e
            time.sleep(30 * (attempt + 1))
            try:  # the PJRT client may be poisoned after an NRT error;
                import jax  # force a backend re-init (device reset)
                jax.clear_backends()
            except Exception:
                pass
    else:
        raise last_exc
    _last_results = res
    return np.concatenate([r["out"].reshape(R) for r in res.results])


def kernel(x, is_train_g, W1, b1, W2, b2, W3, b3):
    x = np.asarray(x, dtype=np.float32)
    args = [np.asarray(W1, np.float32), np.asarray(b1, np.float32),
            np.asarray(W2, np.float32), np.asarray(b2, np.float32),
            np.asarray(W3, np.float32), np.asarray(b3, np.float32)]
    if int(is_train_g):
        R = BATCH // N_CORES
        rows = np.ascontiguousarray(x[:, 0, :])          # [256, 784]
        out = _run(rows, R, _pack_weights(*args))
        return out.reshape(BATCH, 1)
    else:
        R = BATCH * NC_LVL // N_CORES
        rows = np.ascontiguousarray(x.reshape(BATCH * NC_LVL, D_IN))
        out = _run(rows, R, _pack_weights(*args))
        return out.reshape(BATCH, NC_LVL, 1)


# revision 11
# speedup vs baseline: 1.0655x; 1.0655x over previous
"""Trainium2 Bass kernel for nn_DisOrFuncf_34067680591904.

Mathematical note: the reference's output *value* is exactly
fout = sigmoid(MLP(x[:, 0, :])) — the inner/GOGradX machinery only
shapes gradients.  The MLP is 784 -> 512 -> 256 -> 1 with leaky-relu
(0.2) and sigmoid.  Eval path (is_train_g == 0) applies the same MLP to
every (batch, level) row.

Strategy: data parallel — 32 rows/core (train) or 128 (eval); weights
replicated, quantized to fp8-e4m3 (measured end-to-end max rel err
4.5e-3 vs the fp32 reference, ~4x inside the 2e-2 gate).  Scales keep
fp8 values in the normal range: W1*8, W2*4, W3*8; leaky-relu is
positively homogeneous so scales fold into the final sigmoid's `scale`.

Transposed dataflow (d1/d2 kept as [feature, batch]):
  L1  ps1_j[128,R] += w1(j,c).T @ xt_c   j=h1-chunk(4), c=k-chunk(6)
      + bf16 tail chunk (features 768:784 + b1 ones-row)
  lrelu1 on DVE (scalar_mul + max), cast d1t to fp8
  L2  ps2_{j2}[128,R] += w2(j,j2).T @ d1t_j       (fp8)
  lrelu2 + b2 bias on DVE: t=0.2*(ps2+b2); d2=max(ps2+b2, t) -> bf16
  L3  ps3[1,R] += w3_col.T @ d2t_slice (bf16; single-partition output
      row keeps the final 128B store on one DMA engine)
  sigmoid on ACT with scale=1/256, bias=b3

DMA layout (perfetto-driven): the two HWDGE queues stream ~90-200 GB/s
each, so the bulk fp8 bytes are split into one large transfer per
queue; the tiny bf16 tail/const tiles ride the SWDGE (gpsimd) queue
and land early so no accumulation group waits on them.
  sync  : fa = [xt | w1_j0 | w1_j1 | w2_a]     (~287 KB)
  scalar: fb = [w1_j2 | w1_j3 | w2_b]          (~262 KB)
  gpsimd: tl = [w1t | xtt] (18.5 KB), cst (1.3 KB)
"""

import os as _os

import numpy as np
import ml_dtypes

N_CORES = 8
BATCH, NC_LVL, D_IN, D_H1, D_H2 = 256, 4, 784, 512, 256
N_WARM = int(_os.environ.get("KERNEL_N_WARM", "4"))

_compiled = {}  # rows_per_core -> nc


def _build_nc(R: int):
    import concourse.bacc as bacc
    import concourse.tile as tile
    from concourse import mybir

    f32 = mybir.dt.float32
    bf16 = mybir.dt.bfloat16
    fp8 = mybir.dt.float8e4
    nc = bacc.Bacc("TRN2", target_bir_lowering=False, debug=False,
                   num_devices=N_CORES, enable_partition_id=False)

    # Three-stage bulk ladder, all on the sync HWDGE queue (the SDMA
    # engines drain one queue FIFO; the sibling-core 2:1 SDMA mux caps
    # the stream at ~216 GB/s, so ordering is what matters):
    #   fa: [xt | w1_j0 | w1_j1 | tl bytes | cst | cstf | b2r bytes]
    #   fb: [w1_j2 | w1_j3]     fc: [w2]  (small last rung — the only
    #   bytes the L2 tail waits on)
    T0 = 6 * R + 2 * 768                 # tl: [17, 512+R] bf16 as bytes
    C0 = T0 + 2 * (512 + R)              # cst: [128, 2] bf16 as bytes
    F0 = C0 + 4                          # cstf: [128, 3] f32 as bytes
    B0 = F0 + 12                         # b2r: [1, 256] bf16 as bytes
    FA = B0 + 512
    assert T0 % 4 == 0 and C0 % 4 == 0 and F0 % 4 == 0 and B0 % 4 == 0
    fa_d = nc.dram_tensor("fa", [128, FA], fp8, kind="ExternalInput")
    fb_d = nc.dram_tensor("fb", [128, 1536], fp8, kind="ExternalInput")
    fc_d = nc.dram_tensor("fc", [128, 1024], fp8, kind="ExternalInput")
    out_d = nc.dram_tensor("out", [1, R], f32, kind="ExternalOutput")

    with tile.TileContext(nc) as tc:
        with (
            tc.tile_pool(name="const", bufs=1) as cpool,
            tc.tile_pool(name="work", bufs=2) as wpool,
            tc.tile_pool(name="psum", bufs=1, space="PSUM") as ppool,
        ):
            # ---- PE warm-up: keep the HAM clock gate open while DMAs
            # stream (bf16 dummy matmuls on a memset tile).
            if N_WARM:
                wa = cpool.tile([128, 128], bf16, tag="warm_a")
                nc.vector.memset(wa[:], 0.0)
                psw = ppool.tile([128, 128], f32, tag="psw")
                for i in range(N_WARM):
                    nc.tensor.matmul(psw[:], wa[:], wa[:],
                                     start=(i == 0), stop=(i == N_WARM - 1))
                wsb = cpool.tile([1, 1], f32, tag="wsb")
                nc.vector.tensor_copy(wsb[:], psw[0:1, 0:1])

            # ---- DMAs ----
            fa = cpool.tile([128, FA], fp8, tag="fa")
            nc.sync.dma_start(out=fa[:], in_=fa_d[:])
            fb = cpool.tile([128, 1536], fp8, tag="fb")
            nc.sync.dma_start(out=fb[:], in_=fb_d[:])
            fc = cpool.tile([128, 1024], fp8, tag="fc")
            nc.sync.dma_start(out=fc[:], in_=fc_d[:])

            xt = fa[:, 0:6 * R]
            w1 = [fa[:, 6 * R:6 * R + 768], fa[:, 6 * R + 768:6 * R + 1536],
                  fb[:, 0:768], fb[:, 768:1536]]
            w2s = fc[:, 0:1024]
            tlball = fa[0:17, T0:T0 + 2 * (512 + R)].bitcast(bf16)
            w1t = tlball[:, 0:512]
            xtt = tlball[:, 512:512 + R]
            cst = fa[:, C0:C0 + 4].bitcast(bf16)
            cstf = fa[:, F0:F0 + 12].bitcast(f32)
            b2r = fa[0:1, B0:B0 + 512].bitcast(bf16)
            ones1 = cpool.tile([1, R], bf16, tag="ones1")
            nc.vector.memset(ones1[:], 1.0)

            def w2(j, j2):
                return w2s[:, 256 * j + 128 * j2:256 * j + 128 * j2 + 128]

            # ---- PSUM tiles ----
            ps1 = [ppool.tile([128, R], f32, tag=f"ps1_{j}", name=f"ps1_{j}")
                   for j in range(4)]
            ps2 = [ppool.tile([128, R], f32, tag=f"ps2_{j2}", name=f"ps2_{j2}")
                   for j2 in range(2)]
            ps3 = ppool.tile([1, R], f32, tag="ps3")

            d1t = [None] * 4

            def l1_chunk(j):
                for c in range(6):
                    nc.tensor.matmul(ps1[j][:],
                                     w1[j][:, 128 * c:128 * c + 128],
                                     xt[:, R * c:R * c + R],
                                     start=(c == 0), stop=False)
                nc.tensor.matmul(ps1[j][:],
                                 w1t[:, 128 * j:128 * j + 128],
                                 xtt[:, 0:R],
                                 start=False, stop=True)

            def lrelu1(j):
                t = wpool.tile([128, R], f32, tag="t1")
                nc.vector.tensor_scalar_mul(t[:], ps1[j][:], 0.2)
                d = cpool.tile([128, R], fp8, tag=f"d1t_{j}",
                               name=f"d1t_{j}")
                nc.vector.tensor_max(d[:], ps1[j][:], t[:])
                d1t[j] = d

            def l2_chunk(j):
                # b2 bias opens each group (b2r lands with the first
                # DMA rung, so this never gates)
                for j2 in range(2):
                    sl = ps2[j2][:]
                    if j == 0:
                        nc.tensor.matmul(sl, b2r[0:1, 128 * j2:128 * j2 + 128],
                                         ones1[:], start=True, stop=False)
                    nc.tensor.matmul(sl, w2(j, j2), d1t[j][:],
                                     start=False, stop=(j == 3))

            # PE program order: all L1 first (chunk arrival-paced),
            # then all L2 — w2 is the last DMA rung, so no L2 matmul
            # may sit ahead of remaining L1 work in the PE stream.
            l1_chunk(0)
            l1_chunk(1)
            lrelu1(0)
            lrelu1(1)
            l1_chunk(2)
            lrelu1(2)
            l1_chunk(3)
            lrelu1(3)
            for j in range(4):
                l2_chunk(j)

            # ---- L2 lrelu (bias already accumulated in PSUM) ----
            d2t = cpool.tile([128, 2 * R], bf16, tag="d2t")
            for j2 in range(2):
                t2 = wpool.tile([128, R], f32, tag="t2")
                nc.vector.tensor_scalar_mul(t2[:], ps2[j2][:], 0.2)
                nc.vector.tensor_max(d2t[:, R * j2:R * j2 + R],
                                     ps2[j2][:], t2[:])

            # ---- L3: ps3[1,R] = sum_o w3[o] * d2t[o,b] ----
            nc.tensor.matmul(ps3[:], cst[:, 0:1], d2t[:, 0:R],
                             start=True, stop=False)
            nc.tensor.matmul(ps3[:], cst[:, 1:2], d2t[:, R:2 * R],
                             start=False, stop=True)

            # ---- sigmoid((1/256) * ps3 + b3) on ACT ----
            ob = cpool.tile([1, R], f32, tag="ob")
            nc.scalar.activation(ob[:], ps3[:],
                                 mybir.ActivationFunctionType.Sigmoid,
                                 bias=cstf[0:1, 0:1], scale=1.0 / 256.0)
            nc.scalar.dma_start(out=out_d[:], in_=ob[:])

    nc.compile()
    return nc


def _get_nc(R: int):
    if R not in _compiled:
        _compiled[R] = _build_nc(R)
    return _compiled[R]


def _pack_weights(W1, b1, W2, b2, W3, b3):
    f = np.float32
    bf = ml_dtypes.bfloat16
    e4 = ml_dtypes.float8_e4m3
    # w1_j[p, 128c + m] = 8*W1[128j + m, 128c + p]
    w1s = (8.0 * W1).astype(f)
    w1js = []
    for j in range(4):
        blk = w1s[128 * j:128 * j + 128, :768].reshape(128, 6, 128)
        w1js.append(np.ascontiguousarray(
            blk.transpose(2, 1, 0).reshape(128, 768)).astype(e4))
    # tl: [w1t | xtt]; w1t[p, 128j + m] = 8*W1[128j + m, 768 + p],
    # row 16 = 8*b1
    w1t = np.empty((17, 512), dtype=bf)
    w1t[:16] = w1s[:, 768:784].T
    w1t[16] = (8.0 * b1).astype(f)
    # w2 [p, 256j + 128j2 + m] = 4*W2[128j2 + m, 128j + p]
    w2s = (4.0 * W2).astype(f)
    w2p = np.empty((128, 1024), dtype=e4)
    for j in range(4):
        for j2 in range(2):
            w2p[:, 256 * j + 128 * j2:256 * j + 128 * j2 + 128] = \
                w2s[128 * j2:128 * j2 + 128, 128 * j:128 * j + 128].T
    fb = np.concatenate([w1js[2], w1js[3]], axis=1)
    fc = w2p
    # cst cols: 0,1 = 8*w3 (bf16); cstf col 0 = b3 (f32); b2r = 32*b2 bf16
    cst = np.empty((128, 2), dtype=bf)
    cst[:, 0] = (8.0 * W3[0, :128]).astype(f)
    cst[:, 1] = (8.0 * W3[0, 128:]).astype(f)
    cstf = np.empty((128, 3), dtype=f)
    cstf[:, 0] = b3[0]
    cstf[:, 1] = 0.0
    cstf[:, 2] = 0.0
    b2r = (32.0 * b2).astype(bf).reshape(1, 256)
    return w1js, w1t, fb, fc, cst, cstf, b2r


def _pack_x(rows_c: np.ndarray, R: int, w1js, w1t, cst, cstf, b2r):
    # fa = [xt | w1_j0 | tl bytes | cst | cstf | b2r bytes]
    # xt[p, cR + b] = x[b, 128c + p] fp8
    # tl = [w1t | xtt]; xtt rows 0:16 = x[:, 768:784].T, row 16 = ones
    e4 = ml_dtypes.float8_e4m3
    bf = ml_dtypes.bfloat16
    T0 = 6 * R + 2 * 768
    C0 = T0 + 2 * (512 + R)
    F0 = C0 + 4
    B0 = F0 + 12
    FA = B0 + 512
    xt = np.ascontiguousarray(
        rows_c[:, :768].reshape(R, 6, 128).transpose(2, 1, 0)
        .reshape(128, 6 * R)).astype(e4)
    fa = np.zeros((128, FA), dtype=e4)
    fa[:, :6 * R] = xt
    fa[:, 6 * R:6 * R + 768] = w1js[0]
    fa[:, 6 * R + 768:T0] = w1js[1]
    tl = np.empty((17, 512 + R), dtype=bf)
    tl[:, :512] = w1t
    tl[:16, 512:] = rows_c[:, 768:784].T
    tl[16, 512:] = 1.0
    u8 = fa.view(np.uint8)
    u8[0:17, T0:C0] = tl.view(np.uint8)
    u8[:, C0:F0] = cst.view(np.uint8)
    u8[:, F0:B0] = cstf.view(np.uint8)
    u8[0:1, B0:FA] = b2r.view(np.uint8)
    return fa


_trace_opts = None   # test harness hook: kwargs for run_bass_kernel_spmd
_last_results = None


def _run(rows: np.ndarray, R: int, weights) -> np.ndarray:
    global _last_results
    import time
    from concourse.bass_utils import run_bass_kernel_spmd

    nc = _get_nc(R)
    w1js, w1t, fb, fc, cst, cstf, b2r = weights
    in_maps = []
    for c in range(N_CORES):
        fa = _pack_x(rows[c * R:(c + 1) * R], R, w1js, w1t, cst, cstf, b2r)
        in_maps.append({"fa": fa, "fb": fb, "fc": fc})
    last_exc = None
    for attempt in range(4):
        try:
            res = run_bass_kernel_spmd(nc, in_maps, list(range(N_CORES)),
                                       **(_trace_opts or {}))
            break
        except Exception as e:  # transient device wedge: wait and retry
            last_exc = e
            time.sleep(30 * (attempt + 1))
            try:  # the PJRT client may be poisoned after an NRT error;
                import jax  # force a backend re-init (device reset)
                jax.clear_backends()
            except Exception:
                pass
    else:
        raise last_exc
    _last_results = res
    return np.concatenate([r["out"].reshape(R) for r in res.results])


def kernel(x, is_train_g, W1, b1, W2, b2, W3, b3):
    x = np.asarray(x, dtype=np.float32)
    args = [np.asarray(W1, np.float32), np.asarray(b1, np.float32),
            np.asarray(W2, np.float32), np.asarray(b2, np.float32),
            np.asarray(W3, np.float32), np.asarray(b3, np.float32)]
    if int(is_train_g):
        R = BATCH // N_CORES
        rows = np.ascontiguousarray(x[:, 0, :])          # [256, 784]
        out = _run(rows, R, _pack_weights(*args))
        return out.reshape(BATCH, 1)
    else:
        R = BATCH * NC_LVL // N_CORES
        rows = np.ascontiguousarray(x.reshape(BATCH * NC_LVL, D_IN))
        out = _run(rows, R, _pack_weights(*args))
        return out.reshape(BATCH, NC_LVL, 1)
